# revision 3
# baseline (speedup 1.0000x reference)
"""Bass/Tile Trainium2 kernel for the additive-attention module.

reference (per batch row b):
    q = hidden_state @ Wa.T + ba                 # [A]
    k = feature_vectors[b] @ Ua.T                # [L, A]
    e = tanh(q + k) @ w                          # [L]
    attn = softmax(e)                            # [L]
    context[b] = attn @ feature_vectors[b]       # [M]

Sharding: data-parallel over batch B=64 -> 8 cores x 8 rows, params
replicated, no collectives. Each core streams its 32 MB feature_vector
shard from HBM exactly once.

Precision: fv pipeline (fv, Ua, tanh output, attn weights) in fp16;
softmax statistics and all accumulations (PSUM) in fp32.

v2 dataflow change vs v1: the [l, m] -> [m, l] transposes of fv that
fed the k-matmul used to run as 512 PE transpose-mode matmuls plus 512
DVE PSUM-evacuation copies; transpose-mode neither warms the PE HAM
clock-gate nor pipelines well, so they cost ~45% of PE busy time. They
are replaced by the DMA xbar transpose (dma_start_transpose): two
instructions per batch row transpose the whole 2 MB on the DMA engines,
overlapped with the HBM loads. The PE now runs a pure fp16 matmul
stream (k, e, weighted-sum), which keeps HAM at K=8/8.

Per-core dataflow (per batch row):
  - fv cast fp32->fp16 during the HBM DMA (SWDGE), natural [l, m] layout
  - xbar-DMA transposes fv into fvT tiled [128(m%128), 2t+mh, 128(l%128)]
  - k-matmul: UaT stationary, fvT strided rhs; ScalarE evacuates with
    fused per-partition bias q[a] and tanh in one ACTIVATE (fp16 out)
  - e = w.T @ tanh(...) on PE; DVE consolidates e into one [1, 4096] row;
    softmax: DRAM-bounce reshape of e -> [128,32], DVE row max, GPSIMD
    cross-partition max, ACT exp with accum_out row sums, GPSIMD sum
  - weighted sum on PE: attn column [128,1] fp16 stationary, natural
    fv tiles streaming; denominator applied at the end in fp32
  - row b's weighted sum is emitted inside row b+1's k/e loop so PE
    never stalls on the softmax tail or on the xbar transpose
"""

import numpy as np

B, R, M, A, L = 64, 512, 256, 256, 4096
NCORES = 8
BLOC = B // NCORES  # 8 batch rows per core
NL = L // 128  # 32 l-chunks of 128
NJG = 8  # j-groups of 512 l-columns
JW = L // NJG  # 512

_CACHE = {}


def _build():
    from contextlib import ExitStack

    import concourse.bacc as bacc
    import concourse.bass as bass
    import concourse.bass_isa as bass_isa
    import concourse.mybir as mybir
    import concourse.tile as tile
    from concourse.masks import make_identity

    f32 = mybir.dt.float32
    f16 = mybir.dt.float16
    AF = mybir.ActivationFunctionType

    nc = bacc.Bacc("TRN2", target_bir_lowering=False, debug=False,
                   num_devices=NCORES)

    hs = nc.dram_tensor("hidden_state", [BLOC, R], f32, kind="ExternalInput").ap()
    fv = nc.dram_tensor("feature_vectors", [BLOC, L, M], f32,
                        kind="ExternalInput").ap()
    Wa = nc.dram_tensor("Wa", [A, R], f32, kind="ExternalInput").ap()
    Ua = nc.dram_tensor("Ua", [A, M], f32, kind="ExternalInput").ap()
    w = nc.dram_tensor("w", [A, 1], f32, kind="ExternalInput").ap()
    ba = nc.dram_tensor("ba", [1, A], f32, kind="ExternalInput").ap()
    ctx_out = nc.dram_tensor("context", [BLOC, M], f32, kind="ExternalOutput").ap()

    with tile.TileContext(nc) as tc, ExitStack() as ctx:
        singles = ctx.enter_context(tc.tile_pool(name="singles", bufs=1))
        ldpool = ctx.enter_context(tc.tile_pool(name="ldpool", bufs=2))
        fvpool = ctx.enter_context(tc.tile_pool(name="fvpool", bufs=3))
        fvtpool = ctx.enter_context(tc.tile_pool(name="fvtpool", bufs=2))
        work = ctx.enter_context(tc.tile_pool(name="work", bufs=3))
        small = ctx.enter_context(tc.tile_pool(name="small", bufs=2))
        ps_k = ctx.enter_context(tc.tile_pool(name="ps_k", bufs=4, space="PSUM"))
        ps_e = ctx.enter_context(tc.tile_pool(name="ps_e", bufs=2, space="PSUM"))
        ps_mm = ctx.enter_context(tc.tile_pool(name="ps_mm", bufs=2, space="PSUM"))
        dram = ctx.enter_context(tc.tile_pool(name="dram", bufs=2, space="DRAM"))

        # fv load (SWDGE f32->f16 cast) + xbar-DMA transpose, split in
        # parts so the transpose chases the arriving chunks. fvT layout:
        # fvT[p, 2t+mh, k] = fv[128t+k, 128mh+p].
        def issue_load_xbar(b, nparts):
            fv_nat = fvpool.tile([128, NL, M], f16, tag="fv", name="fv")
            fvT = fvtpool.tile([128, 2 * NL, 128], f16, tag="fvt", name="fvt")
            tch = NL // nparts
            for p in range(nparts):
                src = bass.AP(tensor=fv.tensor,
                              offset=b * L * M + p * tch * 128 * M,
                              ap=[[M, 128], [128 * M, tch], [1, M]])
                nc.gpsimd.dma_start(out=fv_nat[:, p * tch:(p + 1) * tch, :],
                                    in_=src)
                nc.sync.dma_start_transpose(
                    out=fvT[:, p * 2 * tch:(p + 1) * 2 * tch, :],
                    in_=fv_nat[:, p * tch:(p + 1) * tch, :])
            return fv_nat, fvT

        pend_fv = {0: issue_load_xbar(0, 4)}

        ident = singles.tile([128, 128], f32, tag="ident", name="ident")
        make_identity(nc, ident)

        # ---- parameters into contraction-major layouts ----
        # WaT[rt] [128(r), 256(a)] fp32: WaT[rt][k, a] = Wa[a, 128*rt + k]
        WaT = [singles.tile([128, A], f32, tag=f"WaT{rt}", name=f"WaT{rt}")
               for rt in range(4)]
        for at in range(2):
            wa_nat = ldpool.tile([128, R], f32, tag="ld", name="ld")
            nc.sync.dma_start(out=wa_nat, in_=Wa[at * 128:(at + 1) * 128, :])
            for rt in range(4):
                ps = ps_mm.tile([128, 128], f32, tag="mm", name="mm")
                nc.tensor.transpose(ps, wa_nat[:, rt * 128:(rt + 1) * 128], ident)
                nc.vector.tensor_copy(out=WaT[rt][:, at * 128:(at + 1) * 128],
                                      in_=ps)
        # UaT[mh] [128(m), 256(a)] fp16: UaT[mh][k, a] = Ua[a, 128*mh + k]
        UaT = [singles.tile([128, A], f16, tag=f"UaT{mh}", name=f"UaT{mh}")
               for mh in range(2)]
        for at in range(2):
            ua_nat = ldpool.tile([128, M], f32, tag="ld", name="ld")
            nc.sync.dma_start(out=ua_nat, in_=Ua[at * 128:(at + 1) * 128, :])
            for mh in range(2):
                ps = ps_mm.tile([128, 128], f32, tag="mm", name="mm")
                nc.tensor.transpose(ps, ua_nat[:, mh * 128:(mh + 1) * 128], ident)
                nc.vector.tensor_copy(out=UaT[mh][:, at * 128:(at + 1) * 128],
                                      in_=ps)
        # w as fp16 stationary columns [128, 1] per a-half (cast during DMA)
        w_sb = [singles.tile([128, 1], f16, tag=f"w{ah}", name=f"w{ah}")
                for ah in range(2)]
        for ah in range(2):
            nc.gpsimd.dma_start(out=w_sb[ah], in_=w[ah * 128:(ah + 1) * 128, :])

        # hsT[rt] [128(r), BLOC] fp32
        hsT = [singles.tile([128, BLOC], f32, tag=f"hsT{rt}", name=f"hsT{rt}")
               for rt in range(4)]
        for rt in range(4):
            src = bass.AP(tensor=hs.tensor, offset=rt * 128,
                          ap=[[1, 128], [R, BLOC]])
            nc.sync.dma_start(out=hsT[rt], in_=src)

        # q = hs @ Wa.T + ba   -> [BLOC, A] fp32
        q_ps = ps_mm.tile([BLOC, A], f32, tag="mm", name="mm")
        for rt in range(4):
            nc.tensor.matmul(q_ps, lhsT=hsT[rt], rhs=WaT[rt],
                             start=(rt == 0), stop=(rt == 3))
        ba_b = singles.tile([BLOC, A], f32, tag="ba", name="ba")
        nc.sync.dma_start(out=ba_b,
                          in_=bass.AP(tensor=ba.tensor, offset=0,
                                      ap=[[0, BLOC], [1, A]]))
        q_sb = singles.tile([BLOC, A], f32, tag="q", name="q")
        nc.vector.tensor_add(q_sb, q_ps, ba_b)
        # qT[ah] [128(a), BLOC] fp32
        qT = [singles.tile([128, BLOC], f32, tag=f"qT{ah}", name=f"qT{ah}")
              for ah in range(2)]
        for ah in range(2):
            ps = ps_mm.tile([128, BLOC], f32, tag="mm", name="mm")
            nc.tensor.transpose(ps, q_sb[:, ah * 128:(ah + 1) * 128],
                                ident[:BLOC, :BLOC])
            nc.vector.tensor_copy(out=qT[ah], in_=ps)

        # weighted sum of row b (runs inside row b+1's k/e loop)
        def ws_stage(b, fv_nat, p_t, rz):
            psw = ps_mm.tile([1, M], f32, tag="mm", name="mm")
            for t in range(NL):
                nc.tensor.matmul(psw, lhsT=p_t[:, t:t + 1], rhs=fv_nat[:, t, :],
                                 start=(t == 0), stop=(t == NL - 1))
            ctxs = small.tile([1, M], f32, tag="ctx", name="ctx")
            nc.vector.tensor_scalar_mul(ctxs, psw, rz)
            nc.sync.dma_start(out=ctx_out[b:b + 1, :], in_=ctxs)

        pend_ws = None

        # ---- main per-batch-row pipeline ----
        for b in range(BLOC):
            if b + 1 < BLOC:
                pend_fv[b + 1] = issue_load_xbar(b + 1, 2)
            fv_nat, fvT = pend_fv.pop(b)

            e_sb = small.tile([1, L], f32, tag="e_sb", name="e_sb")
            e_d = dram.tile([L], f32, tag="e_d", name="e_d")

            t_q = {}

            def emit_K(jg):
                t_sb = [work.tile([128, JW], f16, tag=f"t{ah}", name=f"t{ah}")
                        for ah in range(2)]
                for ah in range(2):
                    psk = ps_k.tile([128, JW], f32, tag="kk", name="kk")
                    for mh in range(2):
                        nc.tensor.matmul(
                            psk, lhsT=UaT[mh][:, ah * 128:(ah + 1) * 128],
                            rhs=fvT[:, 8 * jg + mh:8 * jg + mh + 7:2, :],
                            start=(mh == 0), stop=(mh == 1))
                    nc.scalar.activation(out=t_sb[ah], in_=psk, func=AF.Tanh,
                                         bias=qT[ah][:, b:b + 1], scale=1.0)
                t_q[jg] = t_sb

            def emit_E(jg):
                t_sb = t_q.pop(jg)
                pse = ps_e.tile([1, JW], f32, tag="ee", name="ee")
                for ah in range(2):
                    nc.tensor.matmul(pse, lhsT=w_sb[ah], rhs=t_sb[ah],
                                     start=(ah == 0), stop=(ah == 1))
                nc.vector.tensor_copy(out=e_sb[:, jg * JW:(jg + 1) * JW],
                                      in_=pse)

            # k(i) / e(i-1) skew keeps PE off the ScalarE tanh critical
            # path; the previous row's weighted sum slots in early so it
            # overlaps this row's ScalarE/DVE tail instead of PE idle.
            for i in range(NJG + 1):
                if i < NJG:
                    emit_K(i)
                if i >= 1:
                    emit_E(i - 1)
                if i == 1 and pend_ws is not None:
                    ws_stage(*pend_ws)
                    pend_ws = None

            nc.sync.dma_start(
                out=bass.AP(tensor=e_d.tensor, offset=e_d.offset,
                            ap=[[0, 1], [1, L]]),
                in_=e_sb)

            # softmax pieces: e [1, 4096] -> e_t [128, 32] with
            # e_t[p, t] = e[128*t + p] (partition scatter via DRAM bounce)
            e_t = small.tile([128, NL], f32, tag="e_t", name="e_t")
            nc.sync.dma_start(
                out=e_t,
                in_=bass.AP(tensor=e_d.tensor, offset=e_d.offset,
                            ap=[[1, 128], [128, NL]]))
            mrow = small.tile([128, 1], f32, tag="mrow", name="mrow")
            nc.vector.reduce_max(out=mrow, in_=e_t, axis=mybir.AxisListType.X)
            mall = small.tile([128, 1], f32, tag="mall", name="mall")
            nc.gpsimd.partition_all_reduce(mall, mrow, channels=128,
                                           reduce_op=bass_isa.ReduceOp.max)
            negm = small.tile([128, 1], f32, tag="negm", name="negm")
            nc.vector.tensor_scalar_mul(negm, mall, -1.0)
            p_t = small.tile([128, NL], f16, tag="p_t", name="p_t")
            srow = small.tile([128, 1], f32, tag="srow", name="srow")
            nc.scalar.activation(out=p_t, in_=e_t, func=AF.Exp, bias=negm,
                                 scale=1.0, accum_out=srow)
            sall = small.tile([128, 1], f32, tag="sall", name="sall")
            nc.gpsimd.partition_all_reduce(sall, srow, channels=128,
                                           reduce_op=bass_isa.ReduceOp.add)
            rz = small.tile([1, 1], f32, tag="rz", name="rz")
            nc.vector.reciprocal(out=rz, in_=sall[0:1, :])

            pend_ws = (b, fv_nat, p_t, rz)

        ws_stage(*pend_ws)

    nc.compile()
    return nc


def _get_nc():
    if "nc" not in _CACHE:
        _CACHE["nc"] = _build()
    return _CACHE["nc"]


def kernel(hidden_state, feature_vectors, Wa, Ua, w, ba):
    from concourse.bass_utils import run_bass_kernel_spmd

    nc = _get_nc()
    hidden_state = np.ascontiguousarray(hidden_state, dtype=np.float32)
    feature_vectors = np.ascontiguousarray(feature_vectors, dtype=np.float32)
    params = {
        "Wa": np.ascontiguousarray(Wa, dtype=np.float32),
        "Ua": np.ascontiguousarray(Ua, dtype=np.float32),
        "w": np.ascontiguousarray(w, dtype=np.float32),
        "ba": np.ascontiguousarray(ba, dtype=np.float32),
    }
    in_maps = [
        {
            "hidden_state": hidden_state[c * BLOC:(c + 1) * BLOC],
            "feature_vectors": feature_vectors[c * BLOC:(c + 1) * BLOC],
            **params,
        }
        for c in range(NCORES)
    ]
    res = run_bass_kernel_spmd(nc, in_maps, list(range(NCORES)))
    return np.concatenate([res.results[c]["context"] for c in range(NCORES)],
                          axis=0)


# revision 7
# speedup vs baseline: 1.0360x; 1.0360x over previous
"""Bass/Tile Trainium2 kernel for the additive-attention module.

reference (per batch row b):
    q = hidden_state @ Wa.T + ba                 # [A]
    k = feature_vectors[b] @ Ua.T                # [L, A]
    e = tanh(q + k) @ w                          # [L]
    attn = softmax(e)                            # [L]
    context[b] = attn @ feature_vectors[b]       # [M]

Sharding: data-parallel over batch B=64 -> 8 cores x 8 rows, params
replicated, no collectives. Each core streams its 32 MB feature_vector
shard from HBM exactly once.

Precision: fv pipeline (fv, Ua, tanh output, attn weights) in fp16;
softmax statistics and all accumulations (PSUM) in fp32.

v2 dataflow change vs v1: the [l, m] -> [m, l] transposes of fv that
fed the k-matmul used to run as 512 PE transpose-mode matmuls plus 512
DVE PSUM-evacuation copies; transpose-mode neither warms the PE HAM
clock-gate nor pipelines well, so they cost ~45% of PE busy time. They
are replaced by the DMA xbar transpose (dma_start_transpose): two
instructions per batch row transpose the whole 2 MB on the DMA engines,
overlapped with the HBM loads. The PE now runs a pure fp16 matmul
stream (k, e, weighted-sum), which keeps HAM at K=8/8.

Per-core dataflow (per batch row):
  - fv cast fp32->fp16 during the HBM DMA (SWDGE), natural [l, m] layout
  - xbar-DMA transposes fv into fvT tiled [128(m%128), 2t+mh, 128(l%128)]
  - k-matmul: UaT stationary, fvT strided rhs; ScalarE evacuates with
    fused per-partition bias q[a] and tanh in one ACTIVATE (fp16 out)
  - e = w.T @ tanh(...) on PE; DVE consolidates e into one [1, 4096] row;
    softmax: DRAM-bounce reshape of e -> [128,32], DVE row max, GPSIMD
    cross-partition max, ACT exp with accum_out row sums, GPSIMD sum
  - weighted sum on PE: attn column [128,1] fp16 stationary, natural
    fv tiles streaming; denominator applied at the end in fp32
  - row b's weighted sum is emitted inside row b+1's k/e loop so PE
    never stalls on the softmax tail or on the xbar transpose
"""

import numpy as np

B, R, M, A, L = 64, 512, 256, 256, 4096
NCORES = 8
BLOC = B // NCORES  # 8 batch rows per core
NL = L // 128  # 32 l-chunks of 128
NJG = 8  # j-groups of 512 l-columns
JW = L // NJG  # 512

_CACHE = {}


def _build():
    from contextlib import ExitStack

    import concourse.bacc as bacc
    import concourse.bass as bass
    import concourse.bass_isa as bass_isa
    import concourse.mybir as mybir
    import concourse.tile as tile
    from concourse.masks import make_identity

    f32 = mybir.dt.float32
    f16 = mybir.dt.float16
    AF = mybir.ActivationFunctionType

    nc = bacc.Bacc("TRN2", target_bir_lowering=False, debug=False,
                   num_devices=NCORES)

    hs = nc.dram_tensor("hidden_state", [BLOC, R], f32, kind="ExternalInput").ap()
    fv = nc.dram_tensor("feature_vectors", [BLOC, L, M], f32,
                        kind="ExternalInput").ap()
    Wa = nc.dram_tensor("Wa", [A, R], f32, kind="ExternalInput").ap()
    Ua = nc.dram_tensor("Ua", [A, M], f32, kind="ExternalInput").ap()
    w = nc.dram_tensor("w", [A, 1], f32, kind="ExternalInput").ap()
    ba = nc.dram_tensor("ba", [1, A], f32, kind="ExternalInput").ap()
    ctx_out = nc.dram_tensor("context", [BLOC, M], f32, kind="ExternalOutput").ap()

    with tile.TileContext(nc) as tc, ExitStack() as ctx:
        singles = ctx.enter_context(tc.tile_pool(name="singles", bufs=1))
        ldpool = ctx.enter_context(tc.tile_pool(name="ldpool", bufs=2))
        fvpool = ctx.enter_context(tc.tile_pool(name="fvpool", bufs=4))
        fvtpool = ctx.enter_context(tc.tile_pool(name="fvtpool", bufs=2))
        work = ctx.enter_context(tc.tile_pool(name="work", bufs=4))
        small = ctx.enter_context(tc.tile_pool(name="small", bufs=2))
        ps_k = ctx.enter_context(tc.tile_pool(name="ps_k", bufs=4, space="PSUM"))
        ps_e = ctx.enter_context(tc.tile_pool(name="ps_e", bufs=2, space="PSUM"))
        ps_mm = ctx.enter_context(tc.tile_pool(name="ps_mm", bufs=2, space="PSUM"))
        dram = ctx.enter_context(tc.tile_pool(name="dram", bufs=2, space="DRAM"))

        # fv load (SWDGE f32->f16 cast), split in parts so the xbar
        # transpose can chase the arriving chunks for the first rows.
        def issue_load(b, nparts):
            fv_nat = fvpool.tile([128, NL, M], f16, tag="fv", name="fv")
            tch = NL // nparts
            for p in range(nparts):
                src = bass.AP(tensor=fv.tensor,
                              offset=b * L * M + p * tch * 128 * M,
                              ap=[[M, 128], [128 * M, tch], [1, M]])
                nc.gpsimd.dma_start(out=fv_nat[:, p * tch:(p + 1) * tch, :],
                                    in_=src)
            return fv_nat

        # xbar-DMA transpose of a loaded row. DMA_TRANSPOSE occupies the
        # issuing Sync engine for its whole duration, so it is issued a
        # full row ahead (loads run two rows ahead) and never waits.
        # fvT layout: fvT[p, 2t+mh, k] = fv[128t+k, 128mh+p].
        def issue_xbar(fv_nat, nparts):
            fvT = fvtpool.tile([128, 2 * NL, 128], f16, tag="fvt", name="fvt")
            tch = NL // nparts
            for p in range(nparts):
                nc.sync.dma_start_transpose(
                    out=fvT[:, p * 2 * tch:(p + 1) * 2 * tch, :],
                    in_=fv_nat[:, p * tch:(p + 1) * tch, :])
            return fvT

        fv0 = issue_load(0, 4)
        pend_fv = {0: (fv0, issue_xbar(fv0, 4)), 1: (issue_load(1, 2), None)}

        ident = singles.tile([128, 128], f32, tag="ident", name="ident")
        make_identity(nc, ident)

        # ---- parameters into contraction-major layouts ----
        # WaT[rt] [128(r), 256(a)] fp32: WaT[rt][k, a] = Wa[a, 128*rt + k]
        WaT = [singles.tile([128, A], f32, tag=f"WaT{rt}", name=f"WaT{rt}")
               for rt in range(4)]
        for at in range(2):
            wa_nat = ldpool.tile([128, R], f32, tag="ld", name="ld")
            nc.sync.dma_start(out=wa_nat, in_=Wa[at * 128:(at + 1) * 128, :])
            for rt in range(4):
                ps = ps_mm.tile([128, 128], f32, tag="mm", name="mm")
                nc.tensor.transpose(ps, wa_nat[:, rt * 128:(rt + 1) * 128], ident)
                nc.vector.tensor_copy(out=WaT[rt][:, at * 128:(at + 1) * 128],
                                      in_=ps)
        # UaT[mh] [128(m), 256(a)] fp16: UaT[mh][k, a] = Ua[a, 128*mh + k]
        UaT = [singles.tile([128, A], f16, tag=f"UaT{mh}", name=f"UaT{mh}")
               for mh in range(2)]
        for at in range(2):
            ua_nat = ldpool.tile([128, M], f32, tag="ld", name="ld")
            nc.sync.dma_start(out=ua_nat, in_=Ua[at * 128:(at + 1) * 128, :])
            for mh in range(2):
                ps = ps_mm.tile([128, 128], f32, tag="mm", name="mm")
                nc.tensor.transpose(ps, ua_nat[:, mh * 128:(mh + 1) * 128], ident)
                nc.vector.tensor_copy(out=UaT[mh][:, at * 128:(at + 1) * 128],
                                      in_=ps)
        # w as fp16 stationary columns [128, 1] per a-half (cast during DMA)
        w_sb = [singles.tile([128, 1], f16, tag=f"w{ah}", name=f"w{ah}")
                for ah in range(2)]
        for ah in range(2):
            nc.gpsimd.dma_start(out=w_sb[ah], in_=w[ah * 128:(ah + 1) * 128, :])

        # hsT[rt] [128(r), BLOC] fp32
        hsT = [singles.tile([128, BLOC], f32, tag=f"hsT{rt}", name=f"hsT{rt}")
               for rt in range(4)]
        for rt in range(4):
            src = bass.AP(tensor=hs.tensor, offset=rt * 128,
                          ap=[[1, 128], [R, BLOC]])
            nc.sync.dma_start(out=hsT[rt], in_=src)

        # q = hs @ Wa.T + ba   -> [BLOC, A] fp32
        q_ps = ps_mm.tile([BLOC, A], f32, tag="mm", name="mm")
        for rt in range(4):
            nc.tensor.matmul(q_ps, lhsT=hsT[rt], rhs=WaT[rt],
                             start=(rt == 0), stop=(rt == 3))
        ba_b = singles.tile([BLOC, A], f32, tag="ba", name="ba")
        nc.sync.dma_start(out=ba_b,
                          in_=bass.AP(tensor=ba.tensor, offset=0,
                                      ap=[[0, BLOC], [1, A]]))
        q_sb = singles.tile([BLOC, A], f32, tag="q", name="q")
        nc.vector.tensor_add(q_sb, q_ps, ba_b)
        # qT[ah] [128(a), BLOC] fp32
        qT = [singles.tile([128, BLOC], f32, tag=f"qT{ah}", name=f"qT{ah}")
              for ah in range(2)]
        for ah in range(2):
            ps = ps_mm.tile([128, BLOC], f32, tag="mm", name="mm")
            nc.tensor.transpose(ps, q_sb[:, ah * 128:(ah + 1) * 128],
                                ident[:BLOC, :BLOC])
            nc.vector.tensor_copy(out=qT[ah], in_=ps)

        # weighted sum of row b (runs inside row b+1's k/e loop)
        def ws_stage(b, fv_nat, p_t, rz):
            psw = ps_mm.tile([1, M], f32, tag="mm", name="mm")
            for t in range(NL):
                nc.tensor.matmul(psw, lhsT=p_t[:, t:t + 1], rhs=fv_nat[:, t, :],
                                 start=(t == 0), stop=(t == NL - 1))
            ctxs = small.tile([1, M], f32, tag="ctx", name="ctx")
            nc.vector.tensor_scalar_mul(ctxs, psw, rz)
            nc.sync.dma_start(out=ctx_out[b:b + 1, :], in_=ctxs)

        pend_ws = None

        # ---- main per-batch-row pipeline ----
        for b in range(BLOC):
            if b + 2 < BLOC:
                pend_fv[b + 2] = (issue_load(b + 2, 2), None)
            if b + 1 < BLOC:
                nxt_nat, _ = pend_fv[b + 1]
                pend_fv[b + 1] = (nxt_nat, issue_xbar(nxt_nat, 2))
            fv_nat, fvT = pend_fv.pop(b)

            e_sb = small.tile([1, L], f32, tag="e_sb", name="e_sb")
            e_d = dram.tile([L], f32, tag="e_d", name="e_d")

            t_q = {}

            def emit_K(jg):
                t_sb = [work.tile([128, JW], f16, tag=f"t{ah}", name=f"t{ah}")
                        for ah in range(2)]
                for ah in range(2):
                    psk = ps_k.tile([128, JW], f32, tag="kk", name="kk")
                    for mh in range(2):
                        nc.tensor.matmul(
                            psk, lhsT=UaT[mh][:, ah * 128:(ah + 1) * 128],
                            rhs=fvT[:, 8 * jg + mh:8 * jg + mh + 7:2, :],
                            start=(mh == 0), stop=(mh == 1))
                    nc.scalar.activation(out=t_sb[ah], in_=psk, func=AF.Tanh,
                                         bias=qT[ah][:, b:b + 1], scale=1.0)
                t_q[jg] = t_sb

            def emit_E(jg):
                t_sb = t_q.pop(jg)
                pse = ps_e.tile([1, JW], f32, tag="ee", name="ee")
                for ah in range(2):
                    nc.tensor.matmul(pse, lhsT=w_sb[ah], rhs=t_sb[ah],
                                     start=(ah == 0), stop=(ah == 1))
                nc.vector.tensor_copy(out=e_sb[:, jg * JW:(jg + 1) * JW],
                                      in_=pse)

            # k(i) / e(i-2) skew gives ScalarE two j-groups of PE time to
            # finish each tanh; the previous row's weighted sum slots in
            # at i==3, by which time its softmax tail has drained.
            for i in range(NJG + 2):
                if i < NJG:
                    emit_K(i)
                if i >= 2:
                    emit_E(i - 2)
                if i == 3 and pend_ws is not None:
                    ws_stage(*pend_ws)
                    pend_ws = None

            nc.sync.dma_start(
                out=bass.AP(tensor=e_d.tensor, offset=e_d.offset,
                            ap=[[0, 1], [1, L]]),
                in_=e_sb)

            # softmax pieces: e [1, 4096] -> e_t [128, 32] with
            # e_t[p, t] = e[128*t + p] (partition scatter via DRAM bounce)
            e_t = small.tile([128, NL], f32, tag="e_t", name="e_t")
            nc.sync.dma_start(
                out=e_t,
                in_=bass.AP(tensor=e_d.tensor, offset=e_d.offset,
                            ap=[[1, 128], [128, NL]]))
            mrow = small.tile([128, 1], f32, tag="mrow", name="mrow")
            nc.vector.reduce_max(out=mrow, in_=e_t, axis=mybir.AxisListType.X)
            mall = small.tile([128, 1], f32, tag="mall", name="mall")
            nc.gpsimd.partition_all_reduce(mall, mrow, channels=128,
                                           reduce_op=bass_isa.ReduceOp.max)
            negm = small.tile([128, 1], f32, tag="negm", name="negm")
            nc.vector.tensor_scalar_mul(negm, mall, -1.0)
            p_t = small.tile([128, NL], f16, tag="p_t", name="p_t")
            srow = small.tile([128, 1], f32, tag="srow", name="srow")
            nc.scalar.activation(out=p_t, in_=e_t, func=AF.Exp, bias=negm,
                                 scale=1.0, accum_out=srow)
            sall = small.tile([128, 1], f32, tag="sall", name="sall")
            nc.gpsimd.partition_all_reduce(sall, srow, channels=128,
                                           reduce_op=bass_isa.ReduceOp.add)
            rz = small.tile([1, 1], f32, tag="rz", name="rz")
            nc.vector.reciprocal(out=rz, in_=sall[0:1, :])

            pend_ws = (b, fv_nat, p_t, rz)

        ws_stage(*pend_ws)

    nc.compile()
    return nc


def _get_nc():
    if "nc" not in _CACHE:
        _CACHE["nc"] = _build()
    return _CACHE["nc"]


def kernel(hidden_state, feature_vectors, Wa, Ua, w, ba):
    from concourse.bass_utils import run_bass_kernel_spmd

    nc = _get_nc()
    hidden_state = np.ascontiguousarray(hidden_state, dtype=np.float32)
    feature_vectors = np.ascontiguousarray(feature_vectors, dtype=np.float32)
    params = {
        "Wa": np.ascontiguousarray(Wa, dtype=np.float32),
        "Ua": np.ascontiguousarray(Ua, dtype=np.float32),
        "w": np.ascontiguousarray(w, dtype=np.float32),
        "ba": np.ascontiguousarray(ba, dtype=np.float32),
    }
    in_maps = [
        {
            "hidden_state": hidden_state[c * BLOC:(c + 1) * BLOC],
            "feature_vectors": feature_vectors[c * BLOC:(c + 1) * BLOC],
            **params,
        }
        for c in range(NCORES)
    ]
    res = run_bass_kernel_spmd(nc, in_maps, list(range(NCORES)))
    return np.concatenate([res.results[c]["context"] for c in range(NCORES)],
                          axis=0)
